# revision 7
# baseline (speedup 1.0000x reference)
"""Trainium2 Bass kernel for batched symmetric matrix eigenvalue-ReLU.

Computes f(X) = U max(L, eps) U^T for a batch of symmetric 64x64 matrices
without an explicit eigendecomposition, using the identity

    max(l, eps) = 0.5*(l + eps + |l - eps|)
    f(X) = 0.5*(X + eps I) + 0.5*|A|,   A = X - eps I
    |A|  = A * sign(A)

sign(A) is evaluated with a composite of odd quintic polynomials
(greedy-minimax "polar express" schedule), which is pure batched 64x64
matmul work — ideal for the tensor engine. Each 128-partition SBUF tile
holds a PAIR of matrices (top/bottom half); per pair-matmul we issue two
64x64x64 matmuls into opposite 64x64 quadrant groups of the PE array
(tile_position (0,0) and (64,64)) so both run concurrently.

Per-matrix normalization uses s = (sum_i lambda_i^8)^(1/8) = ||A^4||_F^(1/4
... precisely (||A^4||_F^2)^(1/8)), a guaranteed upper bound on |lambda|max
that is ~1.17x tight for this input distribution, computed from the A^2 and
A^4 products that the first quintic step needs anyway.

Batch-parallel across 8 NeuronCores (1024 matrices per core), zero
communication.
"""

import numpy as np

EPS = 1e-4

# Greedy minimax odd-quintic schedules for sign() on [l0, 1] (margin 1e-4).
COEFFS_7 = [
    (8.48103829949467, -25.16139905980959, 18.673477094265458),
    (4.214111227867911, -3.1301559130549386, 0.5827621343692788),
    (4.0979770378866425, -3.0494189231369537, 0.5735786570744),
    (3.683797789428467, -2.7561392327629224, 0.5402623503749359),
    (2.738295377082593, -2.040512580843473, 0.45975443426149015),
    (1.9797052938750903, -1.3625197166806353, 0.3867516310981405),
    (1.875443911014305, -1.2505834968672136, 0.3751393851760464),
]
COEFFS_6 = [
    (8.415716020989708, -24.90337114999073, 18.468195993537563),
    (4.106609300564693, -3.0554409513066365, 0.5742635615228534),
    (3.7116592758930533, -2.776160184941448, 0.542533837096426),
    (2.783283307149528, -2.0766906848678137, 0.46377422718660993),
    (1.9946765688917107, -1.3779367365269897, 0.3883418347203695),
    (1.8755845391524806, -1.2507239071584233, 0.3751391562678839),
]

N_CORES = 8
D = 64  # matrix dim


def _mm_pair(nc, out_psum, lhsT, rhs, J):
    """Per pair j: two concurrent 64x64x64 matmuls (top & bottom quadrants)."""
    for j in range(J):
        lo, hi = 64 * j, 64 * j + 64
        nc.tensor.matmul(
            out_psum[0:64, lo:hi], lhsT[0:64, lo:hi], rhs[0:64, lo:hi],
            start=True, stop=True, tile_position=(0, 0),
        )
        nc.tensor.matmul(
            out_psum[64:128, lo:hi], lhsT[64:128, lo:hi], rhs[64:128, lo:hi],
            start=True, stop=True, tile_position=(64, 64),
        )


def build_program(n_mats, J=8, coeffs=COEFFS_7):
    """Build the single-core Bass program (SPMD across cores)."""
    import concourse.bass as bass
    import concourse.mybir as mybir
    from concourse import bacc
    from concourse.tile import TileContext

    dt = mybir.dt.float32
    AF = mybir.ActivationFunctionType
    OP = mybir.AluOpType

    B = n_mats
    npair = B // 2
    ngroups = npair // J
    assert npair % J == 0
    FW = 64 * J  # free width of a group tile

    nc = bacc.Bacc()
    x = nc.dram_tensor("x", [B, D, D], dt, kind="ExternalInput")
    y = nc.dram_tensor("y", [B, D, D], dt, kind="ExternalOutput")
    # host-provided constants
    ident = nc.dram_tensor("ident", [128, FW], dt, kind="ExternalInput")
    onesb = nc.dram_tensor("onesb", [128, 128], dt, kind="ExternalInput")

    # [G, b, r, j, c]: group tile partition p=(b*64+r), free f=(j*64+c)
    xr = x.rearrange("(G j b) r c -> G b r j c", b=2, j=J)
    yr = y.rearrange("(G j b) r c -> G b r j c", b=2, j=J)

    a1, b1, c1 = coeffs[0]

    with TileContext(nc) as tc:
        with (
            tc.tile_pool(name="const", bufs=1) as constp,
            tc.tile_pool(name="work", bufs=2) as work,
            tc.tile_pool(name="small", bufs=2) as smallp,
            tc.tile_pool(name="psum", bufs=2, space="PSUM") as psum,
        ):
            I_rep = constp.tile([128, FW], dt, tag="irep")
            nc.sync.dma_start(out=I_rep[:], in_=ident[:])
            ones_dma = constp.tile([128, 128], dt, tag="onesd")
            nc.sync.dma_start(out=ones_dma[:], in_=onesb[:])
            ones_t = constp.tile([128, 128], dt, tag="ones")
            nc.vector.tensor_copy(ones_t[:], ones_dma[:])
            Ia1 = constp.tile([128, FW], dt, tag="ia1")
            nc.vector.tensor_scalar_mul(Ia1[:], I_rep[:], float(a1))

            for g in range(ngroups):
                X = work.tile([128, FW], dt, tag="x")
                nc.sync.dma_start(out=X[:], in_=xr[g])
                A = work.tile([128, FW], dt, tag="a")
                # A = X - eps*I = (I_rep * -eps) + X
                nc.vector.scalar_tensor_tensor(
                    A[:], I_rep[:], -EPS, X[:], OP.mult, OP.add)

                # ---- norm phase: Z0 = A^2, W0 = A^4, mom8 = sum W0^2 ----
                Z0p = psum.tile([128, FW], dt, tag="mm0")
                _mm_pair(nc, Z0p, A, A, J)
                Z0s = work.tile([128, FW], dt, tag="zs")
                nc.vector.tensor_copy(Z0s[:], Z0p[:])
                W0p = psum.tile([128, FW], dt, tag="mm1")
                _mm_pair(nc, W0p, Z0s, Z0s, J)
                W0s = work.tile([128, FW], dt, tag="w0s")
                nc.vector.tensor_copy(W0s[:], W0p[:])
                scratch = work.tile([128, FW], dt, tag="scr")
                partials = smallp.tile([128, 1], dt, tag="part")
                nc.vector.scalar_tensor_tensor(
                    scratch[:], W0s[:], 1.0, W0s[:], OP.mult, OP.mult,
                    accum_out=partials[:])
                mom8 = psum.tile([128, 2], dt, tag="mom")
                nc.tensor.matmul(mom8[:, 0:1], ones_t[:], partials[:],
                                 start=True, stop=True)
                rcp = smallp.tile([128, 1], dt, tag="rcp")
                nc.vector.reciprocal(rcp[:], mom8[:, 0:1])
                # q = (1/mom8)^(1/8) = 1/s
                qa = smallp.tile([128, 1], dt, tag="qa")
                qb = smallp.tile([128, 1], dt, tag="qb")
                nc.scalar.sqrt(qa[:], rcp[:])
                nc.scalar.sqrt(qb[:], qa[:])
                q = smallp.tile([128, 1], dt, tag="q")
                nc.scalar.sqrt(q[:], qb[:])
                q2 = smallp.tile([128, 1], dt, tag="q2")
                nc.vector.tensor_mul(q2[:], q[:], q[:])
                q2b = smallp.tile([128, 1], dt, tag="q2b")
                nc.vector.tensor_scalar_mul(q2b[:], q2[:], float(b1))
                q4 = smallp.tile([128, 1], dt, tag="q4")
                nc.vector.tensor_mul(q4[:], q2[:], q2[:])
                q4c = smallp.tile([128, 1], dt, tag="q4c")
                nc.vector.tensor_scalar_mul(q4c[:], q4[:], float(c1))

                # ---- step 1 (reuses Z0, W0): T1 = a1 I + b1 q^2 Z0 + c1 q^4 W0
                t = work.tile([128, FW], dt, tag="t")
                nc.vector.scalar_tensor_tensor(
                    t[:], Z0p[:], q2b[:], Ia1[:], OP.mult, OP.add)
                T1 = work.tile([128, FW], dt, tag="t1")
                nc.vector.scalar_tensor_tensor(
                    T1[:], W0s[:], q4c[:], t[:], OP.mult, OP.add)
                Y1p = psum.tile([128, FW], dt, tag="mm2")
                _mm_pair(nc, Y1p, A, T1, J)
                Y = work.tile([128, FW], dt, tag="y")
                nc.vector.tensor_scalar_mul(Y[:], Y1p[:], q[:])

                # ---- steps 2..K ----
                n_steps = len(coeffs)
                for k, (a, b, c) in enumerate(coeffs[1:]):
                    last = k == n_steps - 2
                    Zp = psum.tile([128, FW], dt, tag="mm0")
                    _mm_pair(nc, Zp, Y, Y, J)
                    ZS = work.tile([128, FW], dt, tag="zs")
                    nc.vector.tensor_scalar_mul(ZS[:], Zp[:], float(c))
                    U = work.tile([128, FW], dt, tag="u")
                    nc.vector.scalar_tensor_tensor(
                        U[:], I_rep[:], float(b / c), Zp[:], OP.mult, OP.add)
                    Vp = psum.tile([128, FW], dt, tag="mm1")
                    _mm_pair(nc, Vp, U, ZS, J)
                    W = work.tile([128, FW], dt, tag="w")
                    nc.vector.scalar_tensor_tensor(
                        W[:], I_rep[:], float(a), Vp[:], OP.mult, OP.add)
                    Yp = psum.tile([128, FW], dt, tag="mm2")
                    _mm_pair(nc, Yp, Y, W, J)
                    Ynew = work.tile([128, FW], dt, tag="y")
                    nc.vector.tensor_scalar_mul(Ynew[:], Yp[:],
                                                0.5 if last else 1.0)
                    Y = Ynew

                # ---- final: out = 0.5 A sign(A) + 0.5 A + eps I ----
                Gp = psum.tile([128, FW], dt, tag="mm0")
                _mm_pair(nc, Gp, A, Y, J)
                t2 = work.tile([128, FW], dt, tag="t")
                nc.vector.scalar_tensor_tensor(
                    t2[:], I_rep[:], EPS, Gp[:], OP.mult, OP.add)
                outs = work.tile([128, FW], dt, tag="o")
                nc.vector.scalar_tensor_tensor(
                    outs[:], A[:], 0.5, t2[:], OP.mult, OP.add)
                nc.sync.dma_start(out=yr[g], in_=outs[:])

    nc.compile()
    return nc


def make_consts(J=8):
    FW = 64 * J
    eye = np.eye(D, dtype=np.float32)
    ident = np.tile(np.concatenate([eye, eye], axis=0), (1, J))  # [128, FW]
    onesb = np.zeros((128, 128), dtype=np.float32)
    onesb[:64, :64] = 1.0
    onesb[64:, 64:] = 1.0
    return ident.astype(np.float32), onesb


_CACHE = {}


def kernel(x: np.ndarray) -> np.ndarray:
    from concourse.bass_utils import run_bass_kernel_spmd

    B = x.shape[0]
    assert B % N_CORES == 0
    bpc = B // N_CORES
    J = 8
    key = (bpc, J)
    if key not in _CACHE:
        _CACHE[key] = build_program(bpc, J=J)
    nc = _CACHE[key]

    ident, onesb = make_consts(J)
    x = np.ascontiguousarray(x, dtype=np.float32)
    shards = x.reshape(N_CORES, bpc, D, D)
    in_maps = [
        {"x": shards[i], "ident": ident, "onesb": onesb}
        for i in range(N_CORES)
    ]
    res = run_bass_kernel_spmd(nc, in_maps, list(range(N_CORES)))
    out = np.concatenate([res.results[i]["y"] for i in range(N_CORES)], axis=0)
    return out.reshape(B, D, D)


if __name__ == "__main__":
    # smoke test on random symmetric input
    rng = np.random.default_rng(0)
    a = rng.standard_normal((N_CORES * 16, D, D), dtype=np.float32)
    xs = 0.5 * (a + a.transpose(0, 2, 1))
    out = kernel(xs)
    print(out.shape, out.dtype)


# revision 10
# speedup vs baseline: 27.9269x; 27.9269x over previous
"""Trainium2 Bass kernel for batched symmetric matrix eigenvalue-ReLU.

Computes f(X) = U max(L, eps) U^T for a batch of symmetric 64x64 matrices
without an explicit eigendecomposition, using the identity

    max(l, eps) = 0.5*(l + eps + |l - eps|)
    f(X) = 0.5*(X + eps I) + 0.5*|A|,   A = X - eps I
    |A|  = A * sign(A)

sign(A) is evaluated with a composite of odd quintic polynomials
(greedy-minimax "polar express" schedule), which is pure batched 64x64
matmul work — ideal for the tensor engine. Each 128-partition SBUF tile
holds a PAIR of matrices (top/bottom half); per pair-matmul we issue two
64x64x64 matmuls into opposite 64x64 quadrant groups of the PE array
(tile_position (0,0) and (64,64)) so both run concurrently.

Per-matrix normalization uses s = (sum_i lambda_i^8)^(1/8) = ||A^4||_F^(1/4
... precisely (||A^4||_F^2)^(1/8)), a guaranteed upper bound on |lambda|max
that is ~1.17x tight for this input distribution, computed from the A^2 and
A^4 products that the first quintic step needs anyway.

Batch-parallel across 8 NeuronCores (1024 matrices per core), zero
communication.
"""

import numpy as np

EPS = 1e-4

# Greedy minimax odd-quintic schedules for sign() on [l0, 1] (margin 1e-4).
COEFFS_7 = [
    (8.48103829949467, -25.16139905980959, 18.673477094265458),
    (4.214111227867911, -3.1301559130549386, 0.5827621343692788),
    (4.0979770378866425, -3.0494189231369537, 0.5735786570744),
    (3.683797789428467, -2.7561392327629224, 0.5402623503749359),
    (2.738295377082593, -2.040512580843473, 0.45975443426149015),
    (1.9797052938750903, -1.3625197166806353, 0.3867516310981405),
    (1.875443911014305, -1.2505834968672136, 0.3751393851760464),
]
COEFFS_6 = [
    (8.415716020989708, -24.90337114999073, 18.468195993537563),
    (4.106609300564693, -3.0554409513066365, 0.5742635615228534),
    (3.7116592758930533, -2.776160184941448, 0.542533837096426),
    (2.783283307149528, -2.0766906848678137, 0.46377422718660993),
    (1.9946765688917107, -1.3779367365269897, 0.3883418347203695),
    (1.8755845391524806, -1.2507239071584233, 0.3751391562678839),
]

N_CORES = 8
D = 64  # matrix dim


def _mm_pair(nc, out_psum, lhsT, rhs, J):
    """Per pair j: two concurrent 64x64x64 matmuls (top & bottom quadrants)."""
    for j in range(J):
        lo, hi = 64 * j, 64 * j + 64
        nc.tensor.matmul(
            out_psum[0:64, lo:hi], lhsT[0:64, lo:hi], rhs[0:64, lo:hi],
            start=True, stop=True, tile_position=(0, 0),
        )
        nc.tensor.matmul(
            out_psum[64:128, lo:hi], lhsT[64:128, lo:hi], rhs[64:128, lo:hi],
            start=True, stop=True, tile_position=(64, 64),
        )


def build_program(n_mats, J=8, coeffs=COEFFS_6):
    """Build the single-core Bass program (SPMD across cores)."""
    import concourse.bass as bass
    import concourse.mybir as mybir
    from concourse import bacc
    from concourse.tile import TileContext

    dt = mybir.dt.float32
    AF = mybir.ActivationFunctionType
    OP = mybir.AluOpType

    B = n_mats
    npair = B // 2
    ngroups = npair // J
    assert npair % J == 0
    FW = 64 * J  # free width of a group tile

    nc = bacc.Bacc()
    x = nc.dram_tensor("x", [B, D, D], dt, kind="ExternalInput")
    y = nc.dram_tensor("y", [B, D, D], dt, kind="ExternalOutput")
    # host-provided constants
    ident = nc.dram_tensor("ident", [128, FW], dt, kind="ExternalInput")
    onesb = nc.dram_tensor("onesb", [128, 128], dt, kind="ExternalInput")

    # [G, b, r, j, c]: group tile partition p=(b*64+r), free f=(j*64+c)
    xr = x.rearrange("(G j b) r c -> G b r j c", b=2, j=J)
    yr = y.rearrange("(G j b) r c -> G b r j c", b=2, j=J)

    a1, b1, c1 = coeffs[0]

    with TileContext(nc) as tc:
        with (
            tc.tile_pool(name="const", bufs=1) as constp,
            tc.tile_pool(name="work", bufs=2) as work,
            tc.tile_pool(name="small", bufs=2) as smallp,
            tc.tile_pool(name="psum", bufs=1, space="PSUM") as psum,
        ):
            I_rep = constp.tile([128, FW], dt, tag="irep")
            nc.sync.dma_start(out=I_rep[:], in_=ident[:])
            ones_dma = constp.tile([128, 128], dt, tag="onesd")
            nc.sync.dma_start(out=ones_dma[:], in_=onesb[:])
            ones_t = constp.tile([128, 128], dt, tag="ones")
            nc.vector.tensor_copy(ones_t[:], ones_dma[:])
            Ia1 = constp.tile([128, FW], dt, tag="ia1")
            nc.vector.tensor_scalar_mul(Ia1[:], I_rep[:], float(a1))

            def group_pipe(g, sl):
                """Generator emitting one group's pipeline; yields between
                PE products so independent groups can interleave on PE."""
                X = work.tile([128, FW], dt, tag=f"x{sl}")
                nc.sync.dma_start(out=X[:], in_=xr[g])
                A = work.tile([128, FW], dt, tag=f"a{sl}")
                # A = X - eps*I = (I_rep * -eps) + X
                nc.vector.scalar_tensor_tensor(
                    A[:], I_rep[:], -EPS, X[:], OP.mult, OP.add)

                # ---- norm phase: Z0 = A^2, W0 = A^4, mom8 = sum W0^2 ----
                Z0p = psum.tile([128, FW], dt, tag=f"mm0_{sl}")
                _mm_pair(nc, Z0p, A, A, J)
                Z0s = work.tile([128, FW], dt, tag=f"zs{sl}")
                nc.vector.tensor_copy(Z0s[:], Z0p[:])
                yield
                W0p = psum.tile([128, FW], dt, tag=f"mm1_{sl}")
                _mm_pair(nc, W0p, Z0s, Z0s, J)
                W0s = work.tile([128, FW], dt, tag=f"w0s{sl}")
                nc.vector.tensor_copy(W0s[:], W0p[:])
                scratch = work.tile([128, FW], dt, tag=f"scr{sl}")
                partials = smallp.tile([128, 1], dt, tag=f"part{sl}")
                nc.vector.scalar_tensor_tensor(
                    scratch[:], W0s[:], 1.0, W0s[:], OP.mult, OP.mult,
                    accum_out=partials[:])
                mom8 = psum.tile([128, 2], dt, tag=f"mom{sl}")
                nc.tensor.matmul(mom8[:, 0:1], ones_t[:], partials[:],
                                 start=True, stop=True)
                rcp = smallp.tile([128, 1], dt, tag=f"rcp{sl}")
                nc.vector.reciprocal(rcp[:], mom8[:, 0:1])
                # q = (1/mom8)^(1/8) = 1/s
                qa = smallp.tile([128, 1], dt, tag=f"qa{sl}")
                qb = smallp.tile([128, 1], dt, tag=f"qb{sl}")
                nc.scalar.sqrt(qa[:], rcp[:])
                nc.scalar.sqrt(qb[:], qa[:])
                q = smallp.tile([128, 1], dt, tag=f"q{sl}")
                nc.scalar.sqrt(q[:], qb[:])
                q2 = smallp.tile([128, 1], dt, tag=f"q2{sl}")
                nc.vector.tensor_mul(q2[:], q[:], q[:])
                q2b = smallp.tile([128, 1], dt, tag=f"q2b{sl}")
                nc.vector.tensor_scalar_mul(q2b[:], q2[:], float(b1))
                q4 = smallp.tile([128, 1], dt, tag=f"q4{sl}")
                nc.vector.tensor_mul(q4[:], q2[:], q2[:])
                q4c = smallp.tile([128, 1], dt, tag=f"q4c{sl}")
                nc.vector.tensor_scalar_mul(q4c[:], q4[:], float(c1))

                # ---- step 1 (reuses Z0, W0): T1 = a1 I + b1 q^2 Z0 + c1 q^4 W0
                t = work.tile([128, FW], dt, tag=f"t{sl}")
                nc.vector.scalar_tensor_tensor(
                    t[:], Z0p[:], q2b[:], Ia1[:], OP.mult, OP.add)
                T1 = work.tile([128, FW], dt, tag=f"t1{sl}")
                nc.vector.scalar_tensor_tensor(
                    T1[:], W0s[:], q4c[:], t[:], OP.mult, OP.add)
                yield
                Y1p = psum.tile([128, FW], dt, tag=f"mm2_{sl}")
                _mm_pair(nc, Y1p, A, T1, J)
                Y = work.tile([128, FW], dt, tag=f"y{sl}")
                nc.vector.tensor_scalar_mul(Y[:], Y1p[:], q[:])
                yield

                # ---- steps 2..K ----
                n_steps = len(coeffs)
                for k, (a, b, c) in enumerate(coeffs[1:]):
                    last = k == n_steps - 2
                    Zp = psum.tile([128, FW], dt, tag=f"mm0_{sl}")
                    _mm_pair(nc, Zp, Y, Y, J)
                    ZS = work.tile([128, FW], dt, tag=f"zs{sl}")
                    nc.vector.tensor_scalar_mul(ZS[:], Zp[:], float(c))
                    U = work.tile([128, FW], dt, tag=f"u{sl}")
                    nc.vector.scalar_tensor_tensor(
                        U[:], I_rep[:], float(b / c), Zp[:], OP.mult, OP.add)
                    yield
                    Vp = psum.tile([128, FW], dt, tag=f"mm1_{sl}")
                    _mm_pair(nc, Vp, U, ZS, J)
                    W = work.tile([128, FW], dt, tag=f"w{sl}")
                    nc.vector.scalar_tensor_tensor(
                        W[:], I_rep[:], float(a), Vp[:], OP.mult, OP.add)
                    yield
                    Yp = psum.tile([128, FW], dt, tag=f"mm2_{sl}")
                    _mm_pair(nc, Yp, Y, W, J)
                    Ynew = work.tile([128, FW], dt, tag=f"y{sl}")
                    nc.vector.tensor_scalar_mul(Ynew[:], Yp[:],
                                                0.5 if last else 1.0)
                    Y = Ynew
                    yield

                # ---- final: out = 0.5 A sign(A) + 0.5 A + eps I ----
                Gp = psum.tile([128, FW], dt, tag=f"mm0_{sl}")
                _mm_pair(nc, Gp, A, Y, J)
                t2 = work.tile([128, FW], dt, tag=f"t{sl}")
                nc.vector.scalar_tensor_tensor(
                    t2[:], I_rep[:], EPS, Gp[:], OP.mult, OP.add)
                outs = work.tile([128, FW], dt, tag=f"o{sl}")
                nc.vector.scalar_tensor_tensor(
                    outs[:], A[:], 0.5, t2[:], OP.mult, OP.add)
                nc.sync.dma_start(out=yr[g], in_=outs[:])

            NI = 2  # groups interleaved in flight
            for sb in range(0, ngroups, NI):
                gens = [group_pipe(sb + i, i) for i in range(min(NI, ngroups - sb))]
                live = list(gens)
                while live:
                    nxt = []
                    for gen in live:
                        try:
                            next(gen)
                            nxt.append(gen)
                        except StopIteration:
                            pass
                    live = nxt

    nc.compile()
    return nc


def make_consts(J=8):
    FW = 64 * J
    eye = np.eye(D, dtype=np.float32)
    ident = np.tile(np.concatenate([eye, eye], axis=0), (1, J))  # [128, FW]
    onesb = np.zeros((128, 128), dtype=np.float32)
    onesb[:64, :64] = 1.0
    onesb[64:, 64:] = 1.0
    return ident.astype(np.float32), onesb


_CACHE = {}


def kernel(x: np.ndarray) -> np.ndarray:
    from concourse.bass_utils import run_bass_kernel_spmd

    B = x.shape[0]
    assert B % N_CORES == 0
    bpc = B // N_CORES
    J = 8
    key = (bpc, J)
    if key not in _CACHE:
        _CACHE[key] = build_program(bpc, J=J)
    nc = _CACHE[key]

    ident, onesb = make_consts(J)
    x = np.ascontiguousarray(x, dtype=np.float32)
    shards = x.reshape(N_CORES, bpc, D, D)
    in_maps = [
        {"x": shards[i], "ident": ident, "onesb": onesb}
        for i in range(N_CORES)
    ]
    res = run_bass_kernel_spmd(nc, in_maps, list(range(N_CORES)))
    out = np.concatenate([res.results[i]["y"] for i in range(N_CORES)], axis=0)
    return out.reshape(B, D, D)


if __name__ == "__main__":
    # smoke test on random symmetric input
    rng = np.random.default_rng(0)
    a = rng.standard_normal((N_CORES * 16, D, D), dtype=np.float32)
    xs = 0.5 * (a + a.transpose(0, 2, 1))
    out = kernel(xs)
    print(out.shape, out.dtype)


# revision 11
# speedup vs baseline: 27.9508x; 1.0009x over previous
"""Trainium2 Bass kernel for batched symmetric matrix eigenvalue-ReLU.

Computes f(X) = U max(L, eps) U^T for a batch of symmetric 64x64 matrices
without an explicit eigendecomposition, using the identity

    max(l, eps) = 0.5*(l + eps + |l - eps|)
    f(X) = 0.5*(X + eps I) + 0.5*|A|,   A = X - eps I
    |A|  = A * sign(A)

sign(A) is evaluated with a composite of odd quintic polynomials
(greedy-minimax "polar express" schedule), which is pure batched 64x64
matmul work — ideal for the tensor engine. Each 128-partition SBUF tile
holds a PAIR of matrices (top/bottom half); per pair-matmul we issue two
64x64x64 matmuls into opposite 64x64 quadrant groups of the PE array
(tile_position (0,0) and (64,64)) so both run concurrently.

Per-matrix normalization uses s = (sum_i lambda_i^8)^(1/8) = ||A^4||_F^(1/4
... precisely (||A^4||_F^2)^(1/8)), a guaranteed upper bound on |lambda|max
that is ~1.17x tight for this input distribution, computed from the A^2 and
A^4 products that the first quintic step needs anyway.

Batch-parallel across 8 NeuronCores (1024 matrices per core), zero
communication.
"""

import numpy as np

EPS = 1e-4

# Greedy minimax odd-quintic schedules for sign() on [l0, 1] (margin 1e-4).
COEFFS_7 = [
    (8.48103829949467, -25.16139905980959, 18.673477094265458),
    (4.214111227867911, -3.1301559130549386, 0.5827621343692788),
    (4.0979770378866425, -3.0494189231369537, 0.5735786570744),
    (3.683797789428467, -2.7561392327629224, 0.5402623503749359),
    (2.738295377082593, -2.040512580843473, 0.45975443426149015),
    (1.9797052938750903, -1.3625197166806353, 0.3867516310981405),
    (1.875443911014305, -1.2505834968672136, 0.3751393851760464),
]
COEFFS_6 = [
    (8.415716020989708, -24.90337114999073, 18.468195993537563),
    (4.106609300564693, -3.0554409513066365, 0.5742635615228534),
    (3.7116592758930533, -2.776160184941448, 0.542533837096426),
    (2.783283307149528, -2.0766906848678137, 0.46377422718660993),
    (1.9946765688917107, -1.3779367365269897, 0.3883418347203695),
    (1.8755845391524806, -1.2507239071584233, 0.3751391562678839),
]

N_CORES = 8
D = 64  # matrix dim


def _mm_pair(nc, out_psum, lhsT, rhs, J):
    """Per pair j: two concurrent 64x64x64 matmuls (top & bottom quadrants)."""
    for j in range(J):
        lo, hi = 64 * j, 64 * j + 64
        nc.tensor.matmul(
            out_psum[0:64, lo:hi], lhsT[0:64, lo:hi], rhs[0:64, lo:hi],
            start=True, stop=True, tile_position=(0, 0),
        )
        nc.tensor.matmul(
            out_psum[64:128, lo:hi], lhsT[64:128, lo:hi], rhs[64:128, lo:hi],
            start=True, stop=True, tile_position=(64, 64),
        )


def build_program(n_mats, J=8, coeffs=COEFFS_6):
    """Build the single-core Bass program (SPMD across cores)."""
    import concourse.bass as bass
    import concourse.mybir as mybir
    from concourse import bacc
    from concourse.tile import TileContext

    dt = mybir.dt.float32
    AF = mybir.ActivationFunctionType
    OP = mybir.AluOpType

    B = n_mats
    npair = B // 2
    ngroups = npair // J
    assert npair % J == 0
    FW = 64 * J  # free width of a group tile

    nc = bacc.Bacc()
    x = nc.dram_tensor("x", [B, D, D], dt, kind="ExternalInput")
    y = nc.dram_tensor("y", [B, D, D], dt, kind="ExternalOutput")
    # host-provided constants
    ident = nc.dram_tensor("ident", [128, FW], dt, kind="ExternalInput")
    onesb = nc.dram_tensor("onesb", [128, 128], dt, kind="ExternalInput")

    # [G, b, r, j, c]: group tile partition p=(b*64+r), free f=(j*64+c)
    xr = x.rearrange("(G j b) r c -> G b r j c", b=2, j=J)
    yr = y.rearrange("(G j b) r c -> G b r j c", b=2, j=J)

    a1, b1, c1 = coeffs[0]

    with TileContext(nc) as tc:
        with (
            tc.tile_pool(name="const", bufs=1) as constp,
            tc.tile_pool(name="work", bufs=2) as work,
            tc.tile_pool(name="small", bufs=2) as smallp,
            tc.tile_pool(name="psum", bufs=1, space="PSUM") as psum,
        ):
            I_rep = constp.tile([128, FW], dt, tag="irep")
            nc.sync.dma_start(out=I_rep[:], in_=ident[:])
            ones_dma = constp.tile([128, 128], dt, tag="onesd")
            nc.sync.dma_start(out=ones_dma[:], in_=onesb[:])
            ones_t = constp.tile([128, 128], dt, tag="ones")
            nc.vector.tensor_copy(ones_t[:], ones_dma[:])
            Ia1 = constp.tile([128, FW], dt, tag="ia1")
            nc.vector.tensor_scalar_mul(Ia1[:], I_rep[:], float(a1))

            def group_pipe(g, sl):
                """Generator emitting one group's pipeline; yields between
                PE products so independent groups can interleave on PE."""
                X = work.tile([128, FW], dt, tag=f"x{sl}")
                nc.sync.dma_start(out=X[:], in_=xr[g])
                A = work.tile([128, FW], dt, tag=f"a{sl}")
                # A = X - eps*I = (I_rep * -eps) + X
                nc.vector.scalar_tensor_tensor(
                    A[:], I_rep[:], -EPS, X[:], OP.mult, OP.add)

                # ---- norm phase: Z0 = A^2, W0 = A^4, mom8 = sum W0^2 ----
                Z0p = psum.tile([128, FW], dt, tag=f"mm0_{sl}")
                _mm_pair(nc, Z0p, A, A, J)
                Z0s = work.tile([128, FW], dt, tag=f"zs{sl}")
                nc.vector.tensor_copy(Z0s[:], Z0p[:])
                yield
                W0p = psum.tile([128, FW], dt, tag=f"mm1_{sl}")
                _mm_pair(nc, W0p, Z0s, Z0s, J)
                W0s = work.tile([128, FW], dt, tag=f"w0s{sl}")
                nc.vector.tensor_copy(W0s[:], W0p[:])
                scratch = work.tile([128, FW], dt, tag=f"scr{sl}")
                partials = smallp.tile([128, 1], dt, tag=f"part{sl}")
                nc.vector.scalar_tensor_tensor(
                    scratch[:], W0s[:], 1.0, W0s[:], OP.mult, OP.mult,
                    accum_out=partials[:])
                mom8 = psum.tile([128, 2], dt, tag=f"mom{sl}")
                nc.tensor.matmul(mom8[:, 0:1], ones_t[:], partials[:],
                                 start=True, stop=True)
                rcp = smallp.tile([128, 1], dt, tag=f"rcp{sl}")
                nc.vector.reciprocal(rcp[:], mom8[:, 0:1])
                # q = (1/mom8)^(1/8) = 1/s
                qa = smallp.tile([128, 1], dt, tag=f"qa{sl}")
                qb = smallp.tile([128, 1], dt, tag=f"qb{sl}")
                nc.scalar.sqrt(qa[:], rcp[:])
                nc.scalar.sqrt(qb[:], qa[:])
                q = smallp.tile([128, 1], dt, tag=f"q{sl}")
                nc.scalar.sqrt(q[:], qb[:])
                q2 = smallp.tile([128, 1], dt, tag=f"q2{sl}")
                nc.vector.tensor_mul(q2[:], q[:], q[:])
                q2b = smallp.tile([128, 1], dt, tag=f"q2b{sl}")
                nc.vector.tensor_scalar_mul(q2b[:], q2[:], float(b1))
                q4 = smallp.tile([128, 1], dt, tag=f"q4{sl}")
                nc.vector.tensor_mul(q4[:], q2[:], q2[:])
                q4c = smallp.tile([128, 1], dt, tag=f"q4c{sl}")
                nc.vector.tensor_scalar_mul(q4c[:], q4[:], float(c1))

                # ---- step 1 (reuses Z0, W0): T1 = a1 I + b1 q^2 Z0 + c1 q^4 W0
                t = work.tile([128, FW], dt, tag=f"t{sl}")
                nc.vector.scalar_tensor_tensor(
                    t[:], Z0p[:], q2b[:], Ia1[:], OP.mult, OP.add)
                T1 = work.tile([128, FW], dt, tag=f"t1{sl}")
                nc.vector.scalar_tensor_tensor(
                    T1[:], W0s[:], q4c[:], t[:], OP.mult, OP.add)
                yield
                Y1p = psum.tile([128, FW], dt, tag=f"mm2_{sl}")
                _mm_pair(nc, Y1p, A, T1, J)
                Y = work.tile([128, FW], dt, tag=f"y{sl}")
                nc.vector.tensor_scalar_mul(Y[:], Y1p[:], q[:])
                yield

                # ---- steps 2..K ----
                n_steps = len(coeffs)
                for k, (a, b, c) in enumerate(coeffs[1:]):
                    last = k == n_steps - 2
                    Zp = psum.tile([128, FW], dt, tag=f"mm0_{sl}")
                    _mm_pair(nc, Zp, Y, Y, J)
                    ZS = work.tile([128, FW], dt, tag=f"zs{sl}")
                    nc.vector.tensor_scalar_mul(ZS[:], Zp[:], float(c))
                    U = work.tile([128, FW], dt, tag=f"u{sl}")
                    nc.vector.scalar_tensor_tensor(
                        U[:], I_rep[:], float(b / c), Zp[:], OP.mult, OP.add)
                    yield
                    Vp = psum.tile([128, FW], dt, tag=f"mm1_{sl}")
                    _mm_pair(nc, Vp, U, ZS, J)
                    W = work.tile([128, FW], dt, tag=f"w{sl}")
                    nc.vector.scalar_tensor_tensor(
                        W[:], I_rep[:], float(a), Vp[:], OP.mult, OP.add)
                    yield
                    Yp = psum.tile([128, FW], dt, tag=f"mm2_{sl}")
                    _mm_pair(nc, Yp, Y, W, J)
                    Ynew = work.tile([128, FW], dt, tag=f"y{sl}")
                    nc.vector.tensor_scalar_mul(Ynew[:], Yp[:],
                                                0.5 if last else 1.0)
                    Y = Ynew
                    yield

                # ---- final: out = 0.5 A sign(A) + 0.5 A + eps I ----
                Gp = psum.tile([128, FW], dt, tag=f"mm0_{sl}")
                _mm_pair(nc, Gp, A, Y, J)
                t2 = work.tile([128, FW], dt, tag=f"t{sl}")
                nc.vector.scalar_tensor_tensor(
                    t2[:], I_rep[:], EPS, Gp[:], OP.mult, OP.add)
                outs = work.tile([128, FW], dt, tag=f"o{sl}")
                nc.vector.scalar_tensor_tensor(
                    outs[:], A[:], 0.5, t2[:], OP.mult, OP.add)
                nc.sync.dma_start(out=yr[g], in_=outs[:])

            NI = 2   # groups interleaved in flight
            STAG = 3  # pipeline offset so the two q-chains don't align
            for sb in range(0, ngroups, NI):
                gens = [group_pipe(sb + i, i) for i in range(min(NI, ngroups - sb))]
                live = []
                for i, gen in enumerate(gens):
                    try:
                        for _ in range(i * STAG):
                            next(gen)
                        live.append(gen)
                    except StopIteration:
                        pass
                while live:
                    nxt = []
                    for gen in live:
                        try:
                            next(gen)
                            nxt.append(gen)
                        except StopIteration:
                            pass
                    live = nxt

    nc.compile()
    return nc


def make_consts(J=8):
    FW = 64 * J
    eye = np.eye(D, dtype=np.float32)
    ident = np.tile(np.concatenate([eye, eye], axis=0), (1, J))  # [128, FW]
    onesb = np.zeros((128, 128), dtype=np.float32)
    onesb[:64, :64] = 1.0
    onesb[64:, 64:] = 1.0
    return ident.astype(np.float32), onesb


_CACHE = {}


def kernel(x: np.ndarray) -> np.ndarray:
    from concourse.bass_utils import run_bass_kernel_spmd

    B = x.shape[0]
    assert B % N_CORES == 0
    bpc = B // N_CORES
    J = 8
    key = (bpc, J)
    if key not in _CACHE:
        _CACHE[key] = build_program(bpc, J=J)
    nc = _CACHE[key]

    ident, onesb = make_consts(J)
    x = np.ascontiguousarray(x, dtype=np.float32)
    shards = x.reshape(N_CORES, bpc, D, D)
    in_maps = [
        {"x": shards[i], "ident": ident, "onesb": onesb}
        for i in range(N_CORES)
    ]
    res = run_bass_kernel_spmd(nc, in_maps, list(range(N_CORES)))
    out = np.concatenate([res.results[i]["y"] for i in range(N_CORES)], axis=0)
    return out.reshape(B, D, D)


if __name__ == "__main__":
    # smoke test on random symmetric input
    rng = np.random.default_rng(0)
    a = rng.standard_normal((N_CORES * 16, D, D), dtype=np.float32)
    xs = 0.5 * (a + a.transpose(0, 2, 1))
    out = kernel(xs)
    print(out.shape, out.dtype)


# revision 15
# speedup vs baseline: 28.1455x; 1.0070x over previous
"""Trainium2 Bass kernel for batched symmetric matrix eigenvalue-ReLU.

Computes f(X) = U max(L, eps) U^T for a batch of symmetric 64x64 matrices
without an explicit eigendecomposition, using the identity

    max(l, eps) = 0.5*(l + eps + |l - eps|)
    f(X) = 0.5*(X + eps I) + 0.5*|A|,   A = X - eps I
    |A|  = A * sign(A)

sign(A) is evaluated with a composite of odd quintic polynomials
(greedy-minimax "polar express" schedule), which is pure batched 64x64
matmul work — ideal for the tensor engine. Each 128-partition SBUF tile
holds a PAIR of matrices (top/bottom half); per pair-matmul we issue two
64x64x64 matmuls into opposite 64x64 quadrant groups of the PE array
(tile_position (0,0) and (64,64)) so both run concurrently.

Per-matrix normalization uses s = (sum_i lambda_i^8)^(1/8) = ||A^4||_F^(1/4
... precisely (||A^4||_F^2)^(1/8)), a guaranteed upper bound on |lambda|max
that is ~1.17x tight for this input distribution, computed from the A^2 and
A^4 products that the first quintic step needs anyway.

Batch-parallel across 8 NeuronCores (1024 matrices per core), zero
communication.
"""

import numpy as np

EPS = 1e-4

# Greedy minimax odd-quintic schedules for sign() on [l0, 1] (margin 1e-4).
COEFFS_7 = [
    (8.48103829949467, -25.16139905980959, 18.673477094265458),
    (4.214111227867911, -3.1301559130549386, 0.5827621343692788),
    (4.0979770378866425, -3.0494189231369537, 0.5735786570744),
    (3.683797789428467, -2.7561392327629224, 0.5402623503749359),
    (2.738295377082593, -2.040512580843473, 0.45975443426149015),
    (1.9797052938750903, -1.3625197166806353, 0.3867516310981405),
    (1.875443911014305, -1.2505834968672136, 0.3751393851760464),
]
COEFFS_6 = [
    (8.415716020989708, -24.90337114999073, 18.468195993537563),
    (4.106609300564693, -3.0554409513066365, 0.5742635615228534),
    (3.7116592758930533, -2.776160184941448, 0.542533837096426),
    (2.783283307149528, -2.0766906848678137, 0.46377422718660993),
    (1.9946765688917107, -1.3779367365269897, 0.3883418347203695),
    (1.8755845391524806, -1.2507239071584233, 0.3751391562678839),
]

N_CORES = 8
D = 64  # matrix dim


def _mm_pair(nc, out_psum, lhsT, rhs, J):
    """Per pair j: two concurrent 64x64x64 matmuls (top & bottom quadrants)."""
    for j in range(J):
        lo, hi = 64 * j, 64 * j + 64
        nc.tensor.matmul(
            out_psum[0:64, lo:hi], lhsT[0:64, lo:hi], rhs[0:64, lo:hi],
            start=True, stop=True, tile_position=(0, 0),
        )
        nc.tensor.matmul(
            out_psum[64:128, lo:hi], lhsT[64:128, lo:hi], rhs[64:128, lo:hi],
            start=True, stop=True, tile_position=(64, 64),
        )


def build_program(n_mats, J=8, coeffs=COEFFS_6, stag=3, wbufs=2):
    """Build the single-core Bass program (SPMD across cores)."""
    import concourse.bass as bass
    import concourse.mybir as mybir
    from concourse import bacc
    from concourse.tile import TileContext

    dt = mybir.dt.float32
    AF = mybir.ActivationFunctionType
    OP = mybir.AluOpType

    B = n_mats
    npair = B // 2
    ngroups = npair // J
    assert npair % J == 0
    FW = 64 * J  # free width of a group tile

    nc = bacc.Bacc()
    x = nc.dram_tensor("x", [B, D, D], dt, kind="ExternalInput")
    y = nc.dram_tensor("y", [B, D, D], dt, kind="ExternalOutput")
    # host-provided constants
    ident = nc.dram_tensor("ident", [128, FW], dt, kind="ExternalInput")
    onesb = nc.dram_tensor("onesb", [128, 128], dt, kind="ExternalInput")

    # [G, b, r, j, c]: group tile partition p=(b*64+r), free f=(j*64+c)
    xr = x.rearrange("(G j b) r c -> G b r j c", b=2, j=J)
    yr = y.rearrange("(G j b) r c -> G b r j c", b=2, j=J)

    a1, b1, c1 = coeffs[0]

    with TileContext(nc) as tc:
        with (
            tc.tile_pool(name="const", bufs=1) as constp,
            tc.tile_pool(name="work", bufs=wbufs) as work,
            tc.tile_pool(name="small", bufs=2) as smallp,
            tc.tile_pool(name="psum", bufs=1, space="PSUM") as psum,
        ):
            I_rep = constp.tile([128, FW], dt, tag="irep")
            nc.sync.dma_start(out=I_rep[:], in_=ident[:])
            ones_dma = constp.tile([128, 128], dt, tag="onesd")
            nc.sync.dma_start(out=ones_dma[:], in_=onesb[:])
            ones_t = constp.tile([128, 128], dt, tag="ones")
            nc.vector.tensor_copy(ones_t[:], ones_dma[:])
            Ia1 = constp.tile([128, FW], dt, tag="ia1")
            nc.vector.tensor_scalar_mul(Ia1[:], I_rep[:], float(a1))

            def group_pipe(g, sl):
                """Generator emitting one group's pipeline; yields between
                PE products so independent groups can interleave on PE."""
                X = work.tile([128, FW], dt, tag=f"x{sl}")
                nc.sync.dma_start(out=X[:], in_=xr[g])
                A = work.tile([128, FW], dt, tag=f"a{sl}")
                # A = X - eps*I = (I_rep * -eps) + X
                nc.vector.scalar_tensor_tensor(
                    A[:], I_rep[:], -EPS, X[:], OP.mult, OP.add)

                # ---- norm phase: Z0 = A^2, W0 = A^4, mom8 = sum W0^2 ----
                Z0p = psum.tile([128, FW], dt, tag=f"mm0_{sl}")
                _mm_pair(nc, Z0p, A, A, J)
                Z0s = work.tile([128, FW], dt, tag=f"zs{sl}")
                nc.vector.tensor_copy(Z0s[:], Z0p[:])
                yield
                W0p = psum.tile([128, FW], dt, tag=f"mm1_{sl}")
                _mm_pair(nc, W0p, Z0s, Z0s, J)
                W0s = work.tile([128, FW], dt, tag=f"w0s{sl}")
                nc.vector.tensor_copy(W0s[:], W0p[:])
                scratch = work.tile([128, FW], dt, tag=f"scr{sl}")
                partials = smallp.tile([128, 1], dt, tag=f"part{sl}")
                nc.vector.scalar_tensor_tensor(
                    scratch[:], W0s[:], 1.0, W0s[:], OP.mult, OP.mult,
                    accum_out=partials[:])
                mom8 = psum.tile([128, 2], dt, tag=f"mom{sl}")
                nc.tensor.matmul(mom8[:, 0:1], ones_t[:], partials[:],
                                 start=True, stop=True)
                rcp = smallp.tile([128, 1], dt, tag=f"rcp{sl}")
                nc.vector.reciprocal(rcp[:], mom8[:, 0:1])
                # q = (1/mom8)^(1/8) = 1/s
                qa = smallp.tile([128, 1], dt, tag=f"qa{sl}")
                qb = smallp.tile([128, 1], dt, tag=f"qb{sl}")
                nc.scalar.sqrt(qa[:], rcp[:])
                nc.scalar.sqrt(qb[:], qa[:])
                q = smallp.tile([128, 1], dt, tag=f"q{sl}")
                nc.scalar.sqrt(q[:], qb[:])
                q2 = smallp.tile([128, 1], dt, tag=f"q2{sl}")
                nc.vector.tensor_mul(q2[:], q[:], q[:])
                q2b = smallp.tile([128, 1], dt, tag=f"q2b{sl}")
                nc.vector.tensor_scalar_mul(q2b[:], q2[:], float(b1))
                q4 = smallp.tile([128, 1], dt, tag=f"q4{sl}")
                nc.vector.tensor_mul(q4[:], q2[:], q2[:])
                q4c = smallp.tile([128, 1], dt, tag=f"q4c{sl}")
                nc.vector.tensor_scalar_mul(q4c[:], q4[:], float(c1))

                # ---- step 1 (reuses Z0, W0): T1 = a1 I + b1 q^2 Z0 + c1 q^4 W0
                t = work.tile([128, FW], dt, tag=f"t{sl}")
                nc.vector.scalar_tensor_tensor(
                    t[:], Z0p[:], q2b[:], Ia1[:], OP.mult, OP.add)
                T1 = work.tile([128, FW], dt, tag=f"t1{sl}")
                nc.vector.scalar_tensor_tensor(
                    T1[:], W0s[:], q4c[:], t[:], OP.mult, OP.add)
                yield
                Y1p = psum.tile([128, FW], dt, tag=f"mm2_{sl}")
                _mm_pair(nc, Y1p, A, T1, J)
                Y = work.tile([128, FW], dt, tag=f"y{sl}")
                nc.vector.tensor_scalar_mul(Y[:], Y1p[:], q[:])
                yield

                # ---- steps 2..K ----
                n_steps = len(coeffs)
                for k, (a, b, c) in enumerate(coeffs[1:]):
                    last = k == n_steps - 2
                    Zp = psum.tile([128, FW], dt, tag=f"mm0_{sl}")
                    _mm_pair(nc, Zp, Y, Y, J)
                    ZS = work.tile([128, FW], dt, tag=f"zs{sl}")
                    nc.vector.tensor_scalar_mul(ZS[:], Zp[:], float(c))
                    U = work.tile([128, FW], dt, tag=f"u{sl}")
                    nc.vector.scalar_tensor_tensor(
                        U[:], I_rep[:], float(b / c), Zp[:], OP.mult, OP.add)
                    yield
                    Vp = psum.tile([128, FW], dt, tag=f"mm1_{sl}")
                    _mm_pair(nc, Vp, U, ZS, J)
                    W = work.tile([128, FW], dt, tag=f"w{sl}")
                    nc.vector.scalar_tensor_tensor(
                        W[:], I_rep[:], float(a), Vp[:], OP.mult, OP.add)
                    yield
                    Yp = psum.tile([128, FW], dt, tag=f"mm2_{sl}")
                    _mm_pair(nc, Yp, Y, W, J)
                    Ynew = work.tile([128, FW], dt, tag=f"y{sl}")
                    nc.vector.tensor_scalar_mul(Ynew[:], Yp[:],
                                                0.5 if last else 1.0)
                    Y = Ynew
                    yield

                # ---- final: out = 0.5 A sign(A) + 0.5 A + eps I ----
                Gp = psum.tile([128, FW], dt, tag=f"mm0_{sl}")
                _mm_pair(nc, Gp, A, Y, J)
                t2 = work.tile([128, FW], dt, tag=f"t{sl}")
                nc.vector.scalar_tensor_tensor(
                    t2[:], I_rep[:], EPS, Gp[:], OP.mult, OP.add)
                outs = work.tile([128, FW], dt, tag=f"o{sl}")
                nc.vector.scalar_tensor_tensor(
                    outs[:], A[:], 0.5, t2[:], OP.mult, OP.add)
                nc.sync.dma_start(out=yr[g], in_=outs[:])

            NI = 2      # groups interleaved in flight
            STAG = stag  # pipeline offset so the two q-chains don't align
            for sb in range(0, ngroups, NI):
                gens = [group_pipe(sb + i, i) for i in range(min(NI, ngroups - sb))]
                live = []
                for i, gen in enumerate(gens):
                    try:
                        for _ in range(i * STAG):
                            next(gen)
                        live.append(gen)
                    except StopIteration:
                        pass
                while live:
                    nxt = []
                    for gen in live:
                        try:
                            next(gen)
                            nxt.append(gen)
                        except StopIteration:
                            pass
                    live = nxt

    nc.compile()
    return nc


def make_consts(J=8):
    FW = 64 * J
    eye = np.eye(D, dtype=np.float32)
    ident = np.tile(np.concatenate([eye, eye], axis=0), (1, J))  # [128, FW]
    onesb = np.zeros((128, 128), dtype=np.float32)
    onesb[:64, :64] = 1.0
    onesb[64:, 64:] = 1.0
    return ident.astype(np.float32), onesb


_CACHE = {}


def kernel(x: np.ndarray) -> np.ndarray:
    from concourse.bass_utils import run_bass_kernel_spmd

    B = x.shape[0]
    assert B % N_CORES == 0
    bpc = B // N_CORES
    J = 8
    key = (bpc, J)
    if key not in _CACHE:
        _CACHE[key] = build_program(bpc, J=J)
    nc = _CACHE[key]

    ident, onesb = make_consts(J)
    x = np.ascontiguousarray(x, dtype=np.float32)
    shards = x.reshape(N_CORES, bpc, D, D)
    in_maps = [
        {"x": shards[i], "ident": ident, "onesb": onesb}
        for i in range(N_CORES)
    ]
    res = run_bass_kernel_spmd(nc, in_maps, list(range(N_CORES)))
    out = np.concatenate([res.results[i]["y"] for i in range(N_CORES)], axis=0)
    return out.reshape(B, D, D)


if __name__ == "__main__":
    # smoke test on random symmetric input
    rng = np.random.default_rng(0)
    a = rng.standard_normal((N_CORES * 16, D, D), dtype=np.float32)
    xs = 0.5 * (a + a.transpose(0, 2, 1))
    out = kernel(xs)
    print(out.shape, out.dtype)
